# revision 1
# baseline (speedup 1.0000x reference)
"""Trainium2 Bass kernel for the masked block-diagonal LSTM net.

Model structure (hardcoded from the problem spec):
  - x_seq [512, 64, 32], recurrent state HID=1088 = 34 blocks x 32.
  - U projections are masked so hidden block j only sees input feature j
    (block 0 additionally sees features 0,1 again via the interaction rows);
    hidden blocks 32,33 receive NO input projection at all.
  - V recurrent matrices are masked block-diagonal -> the 34 blocks evolve
    completely independently through the scan.

Sharding: hidden-block parallel. Cores 0..7 each own 4 input-driven blocks
(128 hidden rows) x the full batch 512. Layout on device is h^T:
[hid on partitions, batch on free dim], so the recurrent matmul, the gate
activations and the state updates all run at full 128-partition width with
N=512 columns and no transposes anywhere.

Blocks 32,33 are bias-only (no x dependence): their state is identical for
every batch element, so their scalar contribution to the readout (and the
tiny 16-feature static MLP + final sigmoid) is folded into the host-side
unshard step.
"""

import sys

sys.path.insert(0, "/opt/trn_rl_repo")

import numpy as np

B = 512
T = 64
INPUT_SZ = 32
HPF = 32
INTER = [(0, 1), (2, 3)]
NB = INPUT_SZ + len(INTER)  # 34
HID = NB * HPF  # 1088
IN_SZ = INPUT_SZ + 2 * len(INTER)  # 36
F_STAT = 16
N_CORES = 8
BLOCKS_PER_CORE = 4
CORE_HID = BLOCKS_PER_CORE * HPF  # 128
CHUNKS = 2  # batch-column chunks per step (pipelining granularity)
CB = B // CHUNKS

_CACHE = {}


def _build_masks():
    um = np.zeros((IN_SZ, HID), np.float32)
    for i in range(INPUT_SZ):
        um[i, i * HPF : (i + 1) * HPF] = 1.0
    for i in range(0, len(INTER), 2):
        um[i + INPUT_SZ, i * HPF : (i + 1) * HPF] = 1.0
        um[i + INPUT_SZ + 1, i * HPF : (i + 1) * HPF] = 1.0
    vm = np.kron(np.eye(NB, dtype=np.float32), np.ones((HPF, HPF), np.float32))
    return um, vm


def _build_program(repeat=1, loop_n=0):
    # repeat>1 duplicates the whole computation serially (same I/O).
    # loop_n>0 instead wraps ONE copy in a hardware For_i loop executing
    # loop_n times: program size stays constant, so wall-clock deltas
    # between two loop_n values isolate true device execution time from
    # the per-call NEFF dispatch overhead (which scales with program size).
    import concourse.bass as bass
    import concourse.tile as tile
    from concourse import bacc, mybir
    from contextlib import nullcontext

    f32 = mybir.dt.float32
    f16 = mybir.dt.float16
    ACT = mybir.ActivationFunctionType

    nc = bacc.Bacc("TRN2", target_bir_lowering=False, debug=False)

    xf_d = nc.dram_tensor("xf", [5, T * B], f16, kind="ExternalInput").ap()
    wu_d = nc.dram_tensor("wu", [4, 5, CORE_HID], f16, kind="ExternalInput").ap()
    wv_d = nc.dram_tensor("wv", [4, CORE_HID, CORE_HID], f16, kind="ExternalInput").ap()
    oc_d = nc.dram_tensor("oc", [CORE_HID, 1], f16, kind="ExternalInput").ap()
    part_d = nc.dram_tensor("partial", [1, B], f32, kind="ExternalOutput").ap()

    with tile.TileContext(nc) as tc:
        with (
            tc.tile_pool(name="const", bufs=1) as cpool,
            tc.tile_pool(name="state", bufs=2) as spool,
            tc.tile_pool(name="work", bufs=3) as wpool,
            tc.tile_pool(name="psum", bufs=2, space="PSUM") as ppool,
        ):
            xf = cpool.tile([5, T * B], f16, tag="xf")
            nc.sync.dma_start(xf[:], xf_d[:])
            wu = []
            wv = []
            for g in range(4):
                wut = cpool.tile([5, CORE_HID], f16, tag=f"wu{g}")
                nc.sync.dma_start(wut[:], wu_d[g])
                wu.append(wut)
                wvt = cpool.tile([CORE_HID, CORE_HID], f16, tag=f"wv{g}")
                nc.sync.dma_start(wvt[:], wv_d[g])
                wv.append(wvt)
            oc = cpool.tile([CORE_HID, 1], f16, tag="oc")
            nc.sync.dma_start(oc[:], oc_d[:])

            loop_cm = (lambda: tc.For_i(0, loop_n, 1)) if loop_n else None
            for rep in range(repeat):
              with loop_cm() if loop_cm else nullcontext():
                # per-chunk state tiles -> exact dependency granularity
                hs_t = []
                cs_t = []
                for ch in range(CHUNKS):
                    h0 = spool.tile([CORE_HID, CB], f16, tag=f"h{ch}")
                    c0 = spool.tile([CORE_HID, CB], f32, tag=f"c{ch}")
                    nc.vector.memset(h0[:].bitcast(mybir.dt.uint16), 0)
                    nc.vector.memset(c0[:], 0.0)
                    hs_t.append(h0)
                    cs_t.append(c0)

                for t in range(T):
                    for ch in range(CHUNKS):
                        h, c = hs_t[ch], cs_t[ch]
                        # own psum tile per chunk: [128, 4 gates, CB]
                        ps = ppool.tile([128, 4, CB], f32, tag=f"ps{ch}")
                        for g in (0, 1, 2, 3):
                            out = ps[:, g]
                            nc.tensor.matmul(
                                out,
                                wu[g][:],
                                xf[:, t * B + ch * CB : t * B + (ch + 1) * CB],
                                start=True,
                                stop=False,
                            )
                            nc.tensor.matmul(
                                out, wv[g][:], h[:], start=False, stop=True
                            )
                        # one fused sigmoid over all 4 gate banks; the cell
                        # gate's weights are pre-scaled x2 so bank 3 yields
                        # g' = sigmoid(2y) with tanh(y) = 2g' - 1
                        ifog = wpool.tile([CORE_HID, 4, CB], f16, tag=f"ifog{ch}")
                        nc.scalar.activation(ifog[:], ps[:], ACT.Sigmoid)
                        i_, f_, o_, g_ = (ifog[:, k] for k in range(4))
                        # c_new = f*c + (2*(i*g') - i)
                        t1 = wpool.tile([CORE_HID, CB], f16, tag=f"t1{ch}")
                        nc.gpsimd.tensor_mul(t1[:], f_, c[:])  # f*c on Pool
                        t2 = wpool.tile([CORE_HID, CB], f16, tag=f"t2{ch}")
                        nc.vector.tensor_mul(t2[:], i_, g_)  # i*g'
                        u = wpool.tile([CORE_HID, CB], f16, tag=f"u{ch}")
                        nc.vector.scalar_tensor_tensor(
                            u[:], t2[:], 2.0, i_, mybir.AluOpType.mult,
                            mybir.AluOpType.subtract,
                        )
                        c_new = spool.tile([CORE_HID, CB], f16, tag=f"c{ch}")
                        nc.vector.tensor_add(c_new[:], t1[:], u[:])
                        # tanh(c) = 2*sigmoid(2c) - 1
                        sc = wpool.tile([CORE_HID, CB], f16, tag=f"sc{ch}")
                        nc.scalar.activation(sc[:], c_new[:], ACT.Sigmoid, scale=2.0)
                        t3 = wpool.tile([CORE_HID, CB], f16, tag=f"t3{ch}")
                        nc.vector.tensor_mul(t3[:], o_, sc[:])  # o*sc
                        h_new = spool.tile([CORE_HID, CB], f16, tag=f"h{ch}")
                        nc.vector.scalar_tensor_tensor(
                            h_new[:], t3[:], 2.0, o_, mybir.AluOpType.mult,
                            mybir.AluOpType.subtract,
                        )
                        hs_t[ch] = h_new
                        cs_t[ch] = c_new

                # readout partial: oc^T @ h  -> [1, B]
                outsb = wpool.tile([1, B], f32, tag="outsb")
                for ch in range(CHUNKS):
                    pr = ppool.tile([128, 4, CB], f32, tag=f"ps{ch}")
                    nc.tensor.matmul(
                        pr[0:1, 0], oc[:], hs_t[ch][:], start=True, stop=True
                    )
                    nc.vector.tensor_copy(outsb[:, ch * CB : (ch + 1) * CB], pr[0:1, 0])
                nc.sync.dma_start(part_d[:], outsb[:])

    nc.compile()
    return nc


def _pack_inputs(inputs):
    um, vm = _build_masks()
    gates = [
        (inputs["U_i"], inputs["V_i"], inputs["b_i"]),
        (inputs["U_f"], inputs["V_f"], inputs["b_f"]),
        (inputs["U_o"], inputs["V_o"], inputs["b_o"]),
        (inputs["U_c"], inputs["V_c"], inputs["b_c"]),
    ]
    Up = [np.asarray(U, np.float32) * um for U, _, _ in gates]
    Vp = [np.asarray(V, np.float32) * vm for _, V, _ in gates]
    bs = [np.asarray(b, np.float32) for _, _, b in gates]
    x_seq = np.asarray(inputs["x_seq"], np.float32)
    out_coef = np.asarray(inputs["out_coef"], np.float32)

    in_maps = []
    for core in range(N_CORES):
        feats = list(range(4 * core, 4 * core + 4))
        hs = slice(CORE_HID * core, CORE_HID * (core + 1))
        xf = np.ones((5, T * B), np.float32)
        # column index = t*B + b
        xf[0:4] = x_seq[:, :, feats].transpose(2, 1, 0).reshape(4, T * B)
        wu = np.zeros((4, 5, CORE_HID), np.float32)
        wv = np.zeros((4, CORE_HID, CORE_HID), np.float32)
        for g in range(4):
            wu[g, 0:4] = Up[g][feats, hs]
            if core == 0:
                # interaction rows 32,33 multiply x0,x1 -> fold into rows 0,1
                wu[g, 0] += Up[g][32, hs]
                wu[g, 1] += Up[g][33, hs]
            wu[g, 4] = bs[g][hs]
            wv[g] = Vp[g][hs, hs]
        # cell gate (idx 3) pre-scaled x2: tanh(y) = 2*sigmoid(2y) - 1
        wu[3] *= 2.0
        wv[3] *= 2.0
        in_maps.append(
            {
                "xf": xf.astype(np.float16),
                "wu": wu.astype(np.float16),
                "wv": wv.astype(np.float16),
                "oc": np.ascontiguousarray(out_coef[hs]).astype(np.float16),
            }
        )
    return in_maps, Vp, bs, out_coef


def _host_tail(inputs, partials, Vp, bs, out_coef):
    """Bias-only blocks 32,33 (batch-independent scalar) + static MLP +
    final sigmoid. All exact model math, done during unshard."""
    aux = slice(32 * HPF, HID)  # hid 1024:1088
    h = np.zeros(2 * HPF, np.float32)
    cst = np.zeros(2 * HPF, np.float32)
    Va = [V[aux, aux] for V in Vp]
    ba = [b[aux] for b in bs]

    def sig(x):
        return 1.0 / (1.0 + np.exp(-x))

    for _ in range(T):
        i_t = sig(ba[0] + h @ Va[0])
        f_t = sig(ba[1] + h @ Va[1])
        o_t = sig(ba[2] + h @ Va[2])
        g_t = np.tanh(ba[3] + h @ Va[3])
        cst = f_t * cst + i_t * g_t
        h = o_t * np.tanh(cst)
    s_aux = float(h @ out_coef[aux, 0])

    x_stat = np.asarray(inputs["x_stat"], np.float32)
    W1 = np.asarray(inputs["W1"], np.float32)
    b1 = np.asarray(inputs["b1"], np.float32)
    W2 = np.asarray(inputs["W2"], np.float32)
    b2 = np.asarray(inputs["b2"], np.float32)
    hid = np.maximum(x_stat[:, :, None] * W1[None] + b1[None], 0.0)
    mlp = sig(np.einsum("bfk,fk->bf", hid, W2) + b2)
    mlp_part = mlp @ out_coef[HID:, 0]

    z = partials.sum(axis=0) + s_aux + mlp_part + float(np.asarray(inputs["out_bias"])[0])
    return sig(z).astype(np.float32).reshape(B, 1)


def kernel(**inputs):
    from concourse.bass_utils import run_bass_kernel_spmd

    if "nc" not in _CACHE:
        _CACHE["nc"] = _build_program()
    nc = _CACHE["nc"]

    in_maps, Vp, bs, out_coef = _pack_inputs(inputs)
    res = run_bass_kernel_spmd(nc, in_maps, core_ids=list(range(N_CORES)))
    partials = np.stack([res.results[c]["partial"][0] for c in range(N_CORES)])
    return _host_tail(inputs, partials, Vp, bs, out_coef)



# revision 3
# speedup vs baseline: 1.5014x; 1.5014x over previous
"""Trainium2 Bass kernel for the masked block-diagonal LSTM net.

Model structure (hardcoded from the problem spec):
  - x_seq [512, 64, 32], recurrent state HID=1088 = 34 blocks x 32.
  - U projections are masked so hidden block j only sees input feature j
    (block 0 additionally sees features 0,1 again via the interaction rows);
    hidden blocks 32,33 receive NO input projection at all.
  - V recurrent matrices are masked block-diagonal -> the 34 blocks evolve
    completely independently through the scan.

Sharding: hidden-block parallel. Cores 0..7 each own 4 input-driven blocks
(128 hidden rows) x the full batch 512. Layout on device is h^T:
[hid on partitions, batch on free dim], so the recurrent matmul, the gate
activations and the state updates all run at full 128-partition width with
N=512 columns and no transposes anywhere.

Blocks 32,33 are bias-only (no x dependence): their state is identical for
every batch element, so their scalar contribution to the readout (and the
tiny 16-feature static MLP + final sigmoid) is folded into the host-side
unshard step.
"""

import sys

sys.path.insert(0, "/opt/trn_rl_repo")

import numpy as np

B = 512
T = 64
INPUT_SZ = 32
HPF = 32
INTER = [(0, 1), (2, 3)]
NB = INPUT_SZ + len(INTER)  # 34
HID = NB * HPF  # 1088
IN_SZ = INPUT_SZ + 2 * len(INTER)  # 36
F_STAT = 16
N_CORES = 8
BLOCKS_PER_CORE = 4
CORE_HID = BLOCKS_PER_CORE * HPF  # 128
CHUNKS = 2  # batch-column chunks per step (pipelining granularity)
CB = B // CHUNKS

_CACHE = {}


def _build_masks():
    um = np.zeros((IN_SZ, HID), np.float32)
    for i in range(INPUT_SZ):
        um[i, i * HPF : (i + 1) * HPF] = 1.0
    for i in range(0, len(INTER), 2):
        um[i + INPUT_SZ, i * HPF : (i + 1) * HPF] = 1.0
        um[i + INPUT_SZ + 1, i * HPF : (i + 1) * HPF] = 1.0
    vm = np.kron(np.eye(NB, dtype=np.float32), np.ones((HPF, HPF), np.float32))
    return um, vm


def _build_program(repeat=1, loop_n=0):
    # repeat>1 duplicates the whole computation serially (same I/O).
    # loop_n>0 instead wraps ONE copy in a hardware For_i loop executing
    # loop_n times: program size stays constant, so wall-clock deltas
    # between two loop_n values isolate true device execution time from
    # the per-call NEFF dispatch overhead (which scales with program size).
    import concourse.bass as bass
    import concourse.tile as tile
    from concourse import bacc, mybir
    from contextlib import nullcontext

    f32 = mybir.dt.float32
    f16 = mybir.dt.float16
    ACT = mybir.ActivationFunctionType

    nc = bacc.Bacc("TRN2", target_bir_lowering=False, debug=False)

    xf_d = nc.dram_tensor("xf", [5, T * B], f16, kind="ExternalInput").ap()
    wu_d = nc.dram_tensor("wu", [4, 5, CORE_HID], f16, kind="ExternalInput").ap()
    wv_d = nc.dram_tensor("wv", [4, CORE_HID, CORE_HID], f16, kind="ExternalInput").ap()
    oc_d = nc.dram_tensor("oc", [CORE_HID, 1], f16, kind="ExternalInput").ap()
    part_d = nc.dram_tensor("partial", [1, B], f32, kind="ExternalOutput").ap()

    with tile.TileContext(nc) as tc:
        with (
            tc.tile_pool(name="const", bufs=1) as cpool,
            tc.tile_pool(name="state", bufs=2) as spool,
            tc.tile_pool(name="work", bufs=3) as wpool,
            tc.tile_pool(name="psum", bufs=2, space="PSUM") as ppool,
        ):
            xf = cpool.tile([5, T * B], f16, tag="xf")
            nc.sync.dma_start(xf[:], xf_d[:])
            wu = []
            wv = []
            for g in range(4):
                wut = cpool.tile([5, CORE_HID], f16, tag=f"wu{g}")
                nc.sync.dma_start(wut[:], wu_d[g])
                wu.append(wut)
                wvt = cpool.tile([CORE_HID, CORE_HID], f16, tag=f"wv{g}")
                nc.sync.dma_start(wvt[:], wv_d[g])
                wv.append(wvt)
            oc = cpool.tile([CORE_HID, 1], f16, tag="oc")
            nc.sync.dma_start(oc[:], oc_d[:])

            loop_cm = (lambda: tc.For_i(0, loop_n, 1)) if loop_n else None
            for rep in range(repeat):
              with loop_cm() if loop_cm else nullcontext():
                # per-chunk state tiles -> exact dependency granularity
                hs_t = []
                cs_t = []
                for ch in range(CHUNKS):
                    h0 = spool.tile([CORE_HID, CB], f16, tag=f"h{ch}")
                    c0 = spool.tile([CORE_HID, CB], f16, tag=f"c{ch}")
                    nc.vector.memset(h0[:].bitcast(mybir.dt.uint16), 0)
                    nc.vector.memset(c0[:].bitcast(mybir.dt.uint16), 0)
                    hs_t.append(h0)
                    cs_t.append(c0)

                for t in range(T):
                    # gate-outer, chunk-inner MM order: each weight set is
                    # loaded into the PE once per step and reused for both
                    # batch chunks (8 LDWEIGHTS/step instead of 16)
                    ps = [
                        ppool.tile([128, 4, CB], f32, tag=f"ps{ch}", name=f"ps{ch}")
                        for ch in range(CHUNKS)
                    ]
                    for g in (0, 1, 2, 3):
                        for ch in range(CHUNKS):
                            nc.tensor.matmul(
                                ps[ch][:, g],
                                wu[g][:],
                                xf[:, t * B + ch * CB : t * B + (ch + 1) * CB],
                                start=True,
                                stop=False,
                            )
                    for g in (0, 1, 2, 3):
                        for ch in range(CHUNKS):
                            nc.tensor.matmul(
                                ps[ch][:, g], wv[g][:], hs_t[ch][:],
                                start=False, stop=True,
                            )
                    for ch in range(CHUNKS):
                        c = cs_t[ch]
                        # one fused sigmoid over all 4 gate banks; the cell
                        # gate's weights are pre-scaled x2 so bank 3 yields
                        # g' = sigmoid(2y) with tanh(y) = 2g' - 1
                        ifog = wpool.tile([CORE_HID, 4, CB], f16, tag=f"ifog{ch}")
                        nc.scalar.activation(ifog[:], ps[ch][:], ACT.Sigmoid)
                        i_, f_, o_, g_ = (ifog[:, k] for k in range(4))
                        # tanh(g) = 2g' - 1 via 4x-mode tensor_scalar
                        tg = wpool.tile([CORE_HID, CB], f16, tag=f"tg{ch}")
                        nc.vector.tensor_scalar(
                            tg[:], g_, 2.0, -1.0,
                            mybir.AluOpType.mult, mybir.AluOpType.add,
                        )
                        t1 = wpool.tile([CORE_HID, CB], f16, tag=f"t1{ch}")
                        nc.vector.tensor_mul(t1[:], f_, c[:])  # f*c
                        t2 = wpool.tile([CORE_HID, CB], f16, tag=f"t2{ch}")
                        nc.vector.tensor_mul(t2[:], i_, tg[:])  # i*tanh(g)
                        c_new = spool.tile([CORE_HID, CB], f16, tag=f"c{ch}")
                        nc.vector.tensor_add(c_new[:], t1[:], t2[:])
                        # tanh(c) = 2*sigmoid(2c) - 1
                        sc = wpool.tile([CORE_HID, CB], f16, tag=f"sc{ch}")
                        nc.scalar.activation(sc[:], c_new[:], ACT.Sigmoid, scale=2.0)
                        tcn = wpool.tile([CORE_HID, CB], f16, tag=f"tc{ch}")
                        nc.vector.tensor_scalar(
                            tcn[:], sc[:], 2.0, -1.0,
                            mybir.AluOpType.mult, mybir.AluOpType.add,
                        )
                        h_new = spool.tile([CORE_HID, CB], f16, tag=f"h{ch}")
                        nc.vector.tensor_mul(h_new[:], o_, tcn[:])  # o*tanh(c)
                        hs_t[ch] = h_new
                        cs_t[ch] = c_new

                # readout partial: oc^T @ h  -> [1, B]
                outsb = wpool.tile([1, B], f32, tag="outsb")
                for ch in range(CHUNKS):
                    pr = ppool.tile([128, 4, CB], f32, tag=f"ps{ch}")
                    nc.tensor.matmul(
                        pr[0:1, 0], oc[:], hs_t[ch][:], start=True, stop=True
                    )
                    nc.vector.tensor_copy(outsb[:, ch * CB : (ch + 1) * CB], pr[0:1, 0])
                nc.sync.dma_start(part_d[:], outsb[:])

    nc.compile()
    return nc


def _pack_inputs(inputs):
    um, vm = _build_masks()
    gates = [
        (inputs["U_i"], inputs["V_i"], inputs["b_i"]),
        (inputs["U_f"], inputs["V_f"], inputs["b_f"]),
        (inputs["U_o"], inputs["V_o"], inputs["b_o"]),
        (inputs["U_c"], inputs["V_c"], inputs["b_c"]),
    ]
    Up = [np.asarray(U, np.float32) * um for U, _, _ in gates]
    Vp = [np.asarray(V, np.float32) * vm for _, V, _ in gates]
    bs = [np.asarray(b, np.float32) for _, _, b in gates]
    x_seq = np.asarray(inputs["x_seq"], np.float32)
    out_coef = np.asarray(inputs["out_coef"], np.float32)

    in_maps = []
    for core in range(N_CORES):
        feats = list(range(4 * core, 4 * core + 4))
        hs = slice(CORE_HID * core, CORE_HID * (core + 1))
        xf = np.ones((5, T * B), np.float32)
        # column index = t*B + b
        xf[0:4] = x_seq[:, :, feats].transpose(2, 1, 0).reshape(4, T * B)
        wu = np.zeros((4, 5, CORE_HID), np.float32)
        wv = np.zeros((4, CORE_HID, CORE_HID), np.float32)
        for g in range(4):
            wu[g, 0:4] = Up[g][feats, hs]
            if core == 0:
                # interaction rows 32,33 multiply x0,x1 -> fold into rows 0,1
                wu[g, 0] += Up[g][32, hs]
                wu[g, 1] += Up[g][33, hs]
            wu[g, 4] = bs[g][hs]
            wv[g] = Vp[g][hs, hs]
        # cell gate (idx 3) pre-scaled x2: tanh(y) = 2*sigmoid(2y) - 1
        wu[3] *= 2.0
        wv[3] *= 2.0
        in_maps.append(
            {
                "xf": xf.astype(np.float16),
                "wu": wu.astype(np.float16),
                "wv": wv.astype(np.float16),
                "oc": np.ascontiguousarray(out_coef[hs]).astype(np.float16),
            }
        )
    return in_maps, Vp, bs, out_coef


def _host_tail(inputs, partials, Vp, bs, out_coef):
    """Bias-only blocks 32,33 (batch-independent scalar) + static MLP +
    final sigmoid. All exact model math, done during unshard."""
    aux = slice(32 * HPF, HID)  # hid 1024:1088
    h = np.zeros(2 * HPF, np.float32)
    cst = np.zeros(2 * HPF, np.float32)
    Va = [V[aux, aux] for V in Vp]
    ba = [b[aux] for b in bs]

    def sig(x):
        return 1.0 / (1.0 + np.exp(-x))

    for _ in range(T):
        i_t = sig(ba[0] + h @ Va[0])
        f_t = sig(ba[1] + h @ Va[1])
        o_t = sig(ba[2] + h @ Va[2])
        g_t = np.tanh(ba[3] + h @ Va[3])
        cst = f_t * cst + i_t * g_t
        h = o_t * np.tanh(cst)
    s_aux = float(h @ out_coef[aux, 0])

    x_stat = np.asarray(inputs["x_stat"], np.float32)
    W1 = np.asarray(inputs["W1"], np.float32)
    b1 = np.asarray(inputs["b1"], np.float32)
    W2 = np.asarray(inputs["W2"], np.float32)
    b2 = np.asarray(inputs["b2"], np.float32)
    hid = np.maximum(x_stat[:, :, None] * W1[None] + b1[None], 0.0)
    mlp = sig(np.einsum("bfk,fk->bf", hid, W2) + b2)
    mlp_part = mlp @ out_coef[HID:, 0]

    z = partials.sum(axis=0) + s_aux + mlp_part + float(np.asarray(inputs["out_bias"])[0])
    return sig(z).astype(np.float32).reshape(B, 1)


def kernel(**inputs):
    from concourse.bass_utils import run_bass_kernel_spmd

    if "nc" not in _CACHE:
        _CACHE["nc"] = _build_program()
    nc = _CACHE["nc"]

    in_maps, Vp, bs, out_coef = _pack_inputs(inputs)
    res = run_bass_kernel_spmd(nc, in_maps, core_ids=list(range(N_CORES)))
    partials = np.stack([res.results[c]["partial"][0] for c in range(N_CORES)])
    return _host_tail(inputs, partials, Vp, bs, out_coef)



# revision 5
# speedup vs baseline: 1.5021x; 1.0005x over previous
"""Trainium2 Bass kernel for the masked block-diagonal LSTM net.

Model structure (hardcoded from the problem spec):
  - x_seq [512, 64, 32], recurrent state HID=1088 = 34 blocks x 32.
  - U projections are masked so hidden block j only sees input feature j
    (block 0 additionally sees features 0,1 again via the interaction rows);
    hidden blocks 32,33 receive NO input projection at all.
  - V recurrent matrices are masked block-diagonal -> the 34 blocks evolve
    completely independently through the scan.

Sharding: hidden-block parallel. Cores 0..7 each own 4 input-driven blocks
(128 hidden rows) x the full batch 512. Layout on device is h^T:
[hid on partitions, batch on free dim], so the recurrent matmul, the gate
activations and the state updates all run at full 128-partition width with
N=512 columns and no transposes anywhere.

Blocks 32,33 are bias-only (no x dependence): their state is identical for
every batch element, so their scalar contribution to the readout (and the
tiny 16-feature static MLP + final sigmoid) is folded into the host-side
unshard step.
"""

import sys

sys.path.insert(0, "/opt/trn_rl_repo")

import numpy as np

B = 512
T = 64
INPUT_SZ = 32
HPF = 32
INTER = [(0, 1), (2, 3)]
NB = INPUT_SZ + len(INTER)  # 34
HID = NB * HPF  # 1088
IN_SZ = INPUT_SZ + 2 * len(INTER)  # 36
F_STAT = 16
N_CORES = 8
BLOCKS_PER_CORE = 4
CORE_HID = BLOCKS_PER_CORE * HPF  # 128
CHUNKS = 2  # batch-column chunks per step (pipelining granularity)
CB = B // CHUNKS

_CACHE = {}


def _build_masks():
    um = np.zeros((IN_SZ, HID), np.float32)
    for i in range(INPUT_SZ):
        um[i, i * HPF : (i + 1) * HPF] = 1.0
    for i in range(0, len(INTER), 2):
        um[i + INPUT_SZ, i * HPF : (i + 1) * HPF] = 1.0
        um[i + INPUT_SZ + 1, i * HPF : (i + 1) * HPF] = 1.0
    vm = np.kron(np.eye(NB, dtype=np.float32), np.ones((HPF, HPF), np.float32))
    return um, vm


def _build_program(repeat=1, loop_n=0):
    # repeat>1 duplicates the whole computation serially (same I/O).
    # loop_n>0 instead wraps ONE copy in a hardware For_i loop executing
    # loop_n times: program size stays constant, so wall-clock deltas
    # between two loop_n values isolate true device execution time from
    # the per-call NEFF dispatch overhead (which scales with program size).
    import concourse.bass as bass
    import concourse.tile as tile
    from concourse import bacc, mybir
    from contextlib import nullcontext

    f32 = mybir.dt.float32
    f16 = mybir.dt.float16
    ACT = mybir.ActivationFunctionType

    nc = bacc.Bacc("TRN2", target_bir_lowering=False, debug=False)

    xf_d = nc.dram_tensor("xf", [5, T * B], f16, kind="ExternalInput").ap()
    wu_d = nc.dram_tensor("wu", [4, 5, CORE_HID], f16, kind="ExternalInput").ap()
    wv_d = nc.dram_tensor("wv", [4, CORE_HID, CORE_HID], f16, kind="ExternalInput").ap()
    oc_d = nc.dram_tensor("oc", [CORE_HID, 1], f16, kind="ExternalInput").ap()
    part_d = nc.dram_tensor("partial", [1, B], f32, kind="ExternalOutput").ap()

    with tile.TileContext(nc) as tc:
        with (
            tc.tile_pool(name="const", bufs=1) as cpool,
            tc.tile_pool(name="state", bufs=2) as spool,
            tc.tile_pool(name="work", bufs=3) as wpool,
            tc.tile_pool(name="psum", bufs=2, space="PSUM") as ppool,
        ):
            # xf split into 4 segments at partition bases 0/32/64/96 so the
            # load uses 20 partitions (4 parallel DMAs) instead of 5, and wu
            # is replicated to each base so matmul lhsT/rhs bases match.
            SEG = 4
            SEGC = T * B // SEG  # columns per segment (16 steps)
            xf = cpool.tile([128, SEGC], f16, tag="xf")
            for s in range(SEG):
                nc.sync.dma_start(
                    xf[32 * s : 32 * s + 5, :],
                    xf_d[:, s * SEGC : (s + 1) * SEGC],
                )
            wu = []  # wu[g][s] -> [5, CORE_HID] view at partition base 32s
            wv = []
            wuT = cpool.tile([128, 4 * CORE_HID], f16, tag="wuT")
            for g in range(4):
                for s in range(SEG):
                    nc.sync.dma_start(
                        wuT[32 * s : 32 * s + 5, g * CORE_HID : (g + 1) * CORE_HID],
                        wu_d[g],
                    )
                wu.append(
                    [
                        wuT[32 * s : 32 * s + 5, g * CORE_HID : (g + 1) * CORE_HID]
                        for s in range(SEG)
                    ]
                )
                wvt = cpool.tile([CORE_HID, CORE_HID], f16, tag=f"wv{g}")
                nc.sync.dma_start(wvt[:], wv_d[g])
                wv.append(wvt)
            oc = cpool.tile([CORE_HID, 1], f16, tag="oc")
            nc.sync.dma_start(oc[:], oc_d[:])

            loop_cm = (lambda: tc.For_i(0, loop_n, 1)) if loop_n else None
            for rep in range(repeat):
              with loop_cm() if loop_cm else nullcontext():
                # per-chunk state tiles -> exact dependency granularity
                hs_t = []
                cs_t = []
                for ch in range(CHUNKS):
                    h0 = spool.tile([CORE_HID, CB], f16, tag=f"h{ch}")
                    c0 = spool.tile([CORE_HID, CB], f16, tag=f"c{ch}")
                    nc.vector.memset(h0[:].bitcast(mybir.dt.uint16), 0)
                    nc.vector.memset(c0[:].bitcast(mybir.dt.uint16), 0)
                    hs_t.append(h0)
                    cs_t.append(c0)

                for t in range(T):
                    # gate-outer, chunk-inner MM order: each weight set is
                    # loaded into the PE once per step and reused for both
                    # batch chunks (8 LDWEIGHTS/step instead of 16)
                    ps = [
                        ppool.tile([128, 4, CB], f32, tag=f"ps{ch}", name=f"ps{ch}")
                        for ch in range(CHUNKS)
                    ]
                    for g in (0, 1, 2, 3):
                        for ch in range(CHUNKS):
                            nc.tensor.matmul(
                                ps[ch][:, g],
                                wu[g][:],
                                xf[:, t * B + ch * CB : t * B + (ch + 1) * CB],
                                start=True,
                                stop=False,
                            )
                    for g in (0, 1, 2, 3):
                        for ch in range(CHUNKS):
                            nc.tensor.matmul(
                                ps[ch][:, g], wv[g][:], hs_t[ch][:],
                                start=False, stop=True,
                            )
                    # phase 1 (both chunks): gate sigmoid + cell update.
                    # phase 2 (both chunks): tanh(c) + hidden update.
                    # Emitting all phase-1 ACT calls before any phase-2 ACT
                    # call keeps the ACT FIFO from stalling on chunk 0's DVE
                    # chain while chunk 1's gates are already ready.
                    ifogs = []
                    for ch in range(CHUNKS):
                        c = cs_t[ch]
                        # one fused sigmoid over all 4 gate banks; the cell
                        # gate's weights are pre-scaled x2 so bank 3 yields
                        # g' = sigmoid(2y) with tanh(y) = 2g' - 1
                        ifog = wpool.tile([CORE_HID, 4, CB], f16, tag=f"ifog{ch}",
                                          name=f"ifog{ch}")
                        nc.scalar.activation(ifog[:], ps[ch][:], ACT.Sigmoid)
                        i_, f_, o_, g_ = (ifog[:, k] for k in range(4))
                        ifogs.append(ifog)
                        # tanh(g) = 2g' - 1 via 4x-mode tensor_scalar
                        tg = wpool.tile([CORE_HID, CB], f16, tag=f"tg{ch}",
                                        name=f"tg{ch}")
                        nc.vector.tensor_scalar(
                            tg[:], g_, 2.0, -1.0,
                            mybir.AluOpType.mult, mybir.AluOpType.add,
                        )
                        t1 = wpool.tile([CORE_HID, CB], f16, tag=f"t1{ch}",
                                        name=f"t1{ch}")
                        nc.vector.tensor_mul(t1[:], f_, c[:])  # f*c
                        t2 = wpool.tile([CORE_HID, CB], f16, tag=f"t2{ch}",
                                        name=f"t2{ch}")
                        nc.vector.tensor_mul(t2[:], i_, tg[:])  # i*tanh(g)
                        c_new = spool.tile([CORE_HID, CB], f16, tag=f"c{ch}",
                                           name=f"c{ch}")
                        nc.vector.tensor_add(c_new[:], t1[:], t2[:])
                        cs_t[ch] = c_new
                    for ch in range(CHUNKS):
                        o_ = ifogs[ch][:, 2]
                        # tanh(c) = 2*sigmoid(2c) - 1
                        sc = wpool.tile([CORE_HID, CB], f16, tag=f"sc{ch}",
                                        name=f"sc{ch}")
                        nc.scalar.activation(
                            sc[:], cs_t[ch][:], ACT.Sigmoid, scale=2.0
                        )
                        tcn = wpool.tile([CORE_HID, CB], f16, tag=f"tc{ch}",
                                         name=f"tc{ch}")
                        nc.vector.tensor_scalar(
                            tcn[:], sc[:], 2.0, -1.0,
                            mybir.AluOpType.mult, mybir.AluOpType.add,
                        )
                        h_new = spool.tile([CORE_HID, CB], f16, tag=f"h{ch}",
                                           name=f"h{ch}")
                        nc.vector.tensor_mul(h_new[:], o_, tcn[:])  # o*tanh(c)
                        hs_t[ch] = h_new

                # readout partial: oc^T @ h  -> [1, B]
                outsb = wpool.tile([1, B], f32, tag="outsb")
                for ch in range(CHUNKS):
                    pr = ppool.tile([128, 4, CB], f32, tag=f"ps{ch}")
                    nc.tensor.matmul(
                        pr[0:1, 0], oc[:], hs_t[ch][:], start=True, stop=True
                    )
                    nc.vector.tensor_copy(outsb[:, ch * CB : (ch + 1) * CB], pr[0:1, 0])
                nc.sync.dma_start(part_d[:], outsb[:])

    nc.compile()
    return nc


def _pack_inputs(inputs):
    um, vm = _build_masks()
    gates = [
        (inputs["U_i"], inputs["V_i"], inputs["b_i"]),
        (inputs["U_f"], inputs["V_f"], inputs["b_f"]),
        (inputs["U_o"], inputs["V_o"], inputs["b_o"]),
        (inputs["U_c"], inputs["V_c"], inputs["b_c"]),
    ]
    Up = [np.asarray(U, np.float32) * um for U, _, _ in gates]
    Vp = [np.asarray(V, np.float32) * vm for _, V, _ in gates]
    bs = [np.asarray(b, np.float32) for _, _, b in gates]
    x_seq = np.asarray(inputs["x_seq"], np.float32)
    out_coef = np.asarray(inputs["out_coef"], np.float32)

    in_maps = []
    for core in range(N_CORES):
        feats = list(range(4 * core, 4 * core + 4))
        hs = slice(CORE_HID * core, CORE_HID * (core + 1))
        xf = np.ones((5, T * B), np.float32)
        # column index = t*B + b
        xf[0:4] = x_seq[:, :, feats].transpose(2, 1, 0).reshape(4, T * B)
        wu = np.zeros((4, 5, CORE_HID), np.float32)
        wv = np.zeros((4, CORE_HID, CORE_HID), np.float32)
        for g in range(4):
            wu[g, 0:4] = Up[g][feats, hs]
            if core == 0:
                # interaction rows 32,33 multiply x0,x1 -> fold into rows 0,1
                wu[g, 0] += Up[g][32, hs]
                wu[g, 1] += Up[g][33, hs]
            wu[g, 4] = bs[g][hs]
            wv[g] = Vp[g][hs, hs]
        # cell gate (idx 3) pre-scaled x2: tanh(y) = 2*sigmoid(2y) - 1
        wu[3] *= 2.0
        wv[3] *= 2.0
        in_maps.append(
            {
                "xf": xf.astype(np.float16),
                "wu": wu.astype(np.float16),
                "wv": wv.astype(np.float16),
                "oc": np.ascontiguousarray(out_coef[hs]).astype(np.float16),
            }
        )
    return in_maps, Vp, bs, out_coef


def _host_tail(inputs, partials, Vp, bs, out_coef):
    """Bias-only blocks 32,33 (batch-independent scalar) + static MLP +
    final sigmoid. All exact model math, done during unshard."""
    aux = slice(32 * HPF, HID)  # hid 1024:1088
    h = np.zeros(2 * HPF, np.float32)
    cst = np.zeros(2 * HPF, np.float32)
    Va = [V[aux, aux] for V in Vp]
    ba = [b[aux] for b in bs]

    def sig(x):
        return 1.0 / (1.0 + np.exp(-x))

    for _ in range(T):
        i_t = sig(ba[0] + h @ Va[0])
        f_t = sig(ba[1] + h @ Va[1])
        o_t = sig(ba[2] + h @ Va[2])
        g_t = np.tanh(ba[3] + h @ Va[3])
        cst = f_t * cst + i_t * g_t
        h = o_t * np.tanh(cst)
    s_aux = float(h @ out_coef[aux, 0])

    x_stat = np.asarray(inputs["x_stat"], np.float32)
    W1 = np.asarray(inputs["W1"], np.float32)
    b1 = np.asarray(inputs["b1"], np.float32)
    W2 = np.asarray(inputs["W2"], np.float32)
    b2 = np.asarray(inputs["b2"], np.float32)
    hid = np.maximum(x_stat[:, :, None] * W1[None] + b1[None], 0.0)
    mlp = sig(np.einsum("bfk,fk->bf", hid, W2) + b2)
    mlp_part = mlp @ out_coef[HID:, 0]

    z = partials.sum(axis=0) + s_aux + mlp_part + float(np.asarray(inputs["out_bias"])[0])
    return sig(z).astype(np.float32).reshape(B, 1)


def kernel(**inputs):
    from concourse.bass_utils import run_bass_kernel_spmd

    if "nc" not in _CACHE:
        _CACHE["nc"] = _build_program()
    nc = _CACHE["nc"]

    in_maps, Vp, bs, out_coef = _pack_inputs(inputs)
    res = run_bass_kernel_spmd(nc, in_maps, core_ids=list(range(N_CORES)))
    partials = np.stack([res.results[c]["partial"][0] for c in range(N_CORES)])
    return _host_tail(inputs, partials, Vp, bs, out_coef)



# revision 11
# speedup vs baseline: 1.7948x; 1.1948x over previous
"""Trainium2 Bass kernel for the masked block-diagonal LSTM net.

Model structure (hardcoded from the problem spec):
  - x_seq [512, 64, 32], recurrent state HID=1088 = 34 blocks x 32.
  - U projections are masked so hidden block j only sees input feature j
    (block 0 additionally sees features 0,1 again via the interaction rows);
    hidden blocks 32,33 receive NO input projection at all.
  - V recurrent matrices are masked block-diagonal -> the 34 blocks evolve
    completely independently through the scan.

Sharding: hidden-block parallel. Cores 0..7 each own 4 input-driven blocks
(128 hidden rows) x the full batch 512. Layout on device is h^T:
[hid on partitions, batch on free dim], so the recurrent matmul, the gate
activations and the state updates all run at full 128-partition width with
N=512 columns and no transposes anywhere.

Blocks 32,33 are bias-only (no x dependence): their state is identical for
every batch element, so their scalar contribution to the readout (and the
tiny 16-feature static MLP + final sigmoid) is folded into the host-side
unshard step.
"""

import sys

sys.path.insert(0, "/opt/trn_rl_repo")

import numpy as np

B = 512
T = 64
INPUT_SZ = 32
HPF = 32
INTER = [(0, 1), (2, 3)]
NB = INPUT_SZ + len(INTER)  # 34
HID = NB * HPF  # 1088
IN_SZ = INPUT_SZ + 2 * len(INTER)  # 36
F_STAT = 16
N_CORES = 8
BLOCKS_PER_CORE = 4
CORE_HID = BLOCKS_PER_CORE * HPF  # 128
CHUNKS = 2  # batch-column chunks per step (pipelining granularity)
CB = B // CHUNKS

_CACHE = {}


def _build_masks():
    um = np.zeros((IN_SZ, HID), np.float32)
    for i in range(INPUT_SZ):
        um[i, i * HPF : (i + 1) * HPF] = 1.0
    for i in range(0, len(INTER), 2):
        um[i + INPUT_SZ, i * HPF : (i + 1) * HPF] = 1.0
        um[i + INPUT_SZ + 1, i * HPF : (i + 1) * HPF] = 1.0
    vm = np.kron(np.eye(NB, dtype=np.float32), np.ones((HPF, HPF), np.float32))
    return um, vm


def _build_program(repeat=1, loop_n=0):
    # repeat>1 duplicates the whole computation serially (same I/O).
    # loop_n>0 instead wraps ONE copy in a hardware For_i loop executing
    # loop_n times: program size stays constant, so wall-clock deltas
    # between two loop_n values isolate true device execution time from
    # the per-call NEFF dispatch overhead (which scales with program size).
    import concourse.bass as bass
    import concourse.tile as tile
    from concourse import bacc, mybir
    from contextlib import nullcontext

    f32 = mybir.dt.float32
    f16 = mybir.dt.float16
    ACT = mybir.ActivationFunctionType

    nc = bacc.Bacc("TRN2", target_bir_lowering=False, debug=False)

    xf_d = nc.dram_tensor("xf", [5, T * B], f16, kind="ExternalInput").ap()
    wu_d = nc.dram_tensor("wu", [4, 5, CORE_HID], f16, kind="ExternalInput").ap()
    wv_d = nc.dram_tensor("wv", [4, CORE_HID, CORE_HID], f16, kind="ExternalInput").ap()
    oc_d = nc.dram_tensor("oc", [CORE_HID, 1], f16, kind="ExternalInput").ap()
    part_d = nc.dram_tensor("partial", [1, B], f32, kind="ExternalOutput").ap()

    with tile.TileContext(nc) as tc:
        with (
            tc.tile_pool(name="const", bufs=1) as cpool,
            tc.tile_pool(name="state", bufs=2) as spool,
            tc.tile_pool(name="work", bufs=3) as wpool,
            tc.tile_pool(name="psum", bufs=2, space="PSUM") as ppool,
        ):
            # xf split into 4 segments at partition bases 0/32/64/96 so the
            # load uses 20 partitions (4 parallel DMAs) instead of 5, and wu
            # is replicated to each base so matmul lhsT/rhs bases match.
            # Weights are DMA'd first (tiny) so compute isn't queued behind
            # the xf bulk; the xf segments are issued from different engines
            # so they overlap each other and the first steps.
            SEG = 4
            SEGC = T * B // SEG  # columns per segment (16 steps)
            wuT = cpool.tile([128, 4 * CORE_HID], f16, tag="wuT")
            wu = [
                [
                    wuT[32 * s : 32 * s + 5, g * CORE_HID : (g + 1) * CORE_HID]
                    for s in range(SEG)
                ]
                for g in range(4)
            ]
            wv = []
            # step-0-critical loads first on SP: segment-0 wu, wv, oc
            for g in range(4):
                nc.sync.dma_start(wu[g][0], wu_d[g])
                wvt = cpool.tile([CORE_HID, CORE_HID], f16, tag=f"wv{g}")
                nc.sync.dma_start(wvt[:], wv_d[g])
                wv.append(wvt)
            oc = cpool.tile([CORE_HID, 1], f16, tag="oc")
            nc.sync.dma_start(oc[:], oc_d[:])
            # xf segment 0 in parallel on the (otherwise idle) Pool engine
            xf = cpool.tile([128, SEGC], f16, tag="xf")
            nc.gpsimd.dma_start(xf[0:5, :], xf_d[:, 0:SEGC])
            # remaining segments stream in on SP behind the critical loads
            for s in range(1, SEG):
                for g in range(4):
                    nc.sync.dma_start(wu[g][s], wu_d[g])
                nc.sync.dma_start(
                    xf[32 * s : 32 * s + 5, :],
                    xf_d[:, s * SEGC : (s + 1) * SEGC],
                )

            loop_cm = (lambda: tc.For_i(0, loop_n, 1)) if loop_n else None
            for rep in range(repeat):
              with loop_cm() if loop_cm else nullcontext():
                # per-chunk state tiles -> exact dependency granularity
                hs_t = []
                cs_t = []
                for ch in range(CHUNKS):
                    h0 = spool.tile([CORE_HID, CB], f16, tag=f"h{ch}")
                    c0 = spool.tile([CORE_HID, CB], f16, tag=f"c{ch}")
                    nc.vector.memset(h0[:].bitcast(mybir.dt.uint16), 0)
                    nc.vector.memset(c0[:].bitcast(mybir.dt.uint16), 0)
                    hs_t.append(h0)
                    cs_t.append(c0)

                for t in range(T):
                    seg, tl = t // 16, t % 16
                    xcols = lambda ch: xf[
                        32 * seg : 32 * seg + 5,
                        tl * B + ch * CB : tl * B + (ch + 1) * CB,
                    ]
                    # gate-outer, chunk-inner MM order: each weight set is
                    # loaded into the PE once per step and reused for both
                    # batch chunks (8 LDWEIGHTS/step instead of 16)
                    ps = [
                        ppool.tile([128, 4, CB], f32, tag=f"ps{ch}", name=f"ps{ch}")
                        for ch in range(CHUNKS)
                    ]
                    for g in (0, 1, 2, 3):
                        for ch in range(CHUNKS):
                            nc.tensor.matmul(
                                ps[ch][:, g], wu[g][seg], xcols(ch),
                                start=True, stop=False,
                                tile_position=(32 * seg, 0),
                            )
                    for g in (0, 1, 2, 3):
                        for ch in range(CHUNKS):
                            nc.tensor.matmul(
                                ps[ch][:, g], wv[g][:], hs_t[ch][:],
                                start=False, stop=True,
                            )
                    # Engine-FIFO-tuned emission: the loop-carried chain is
                    # G_A -> (G_B on ACT) -> S_A -> h_A -> rec MMs -> G_A'.
                    # h_A is emitted before c_B so it doesn't queue behind
                    # chunk B's cell update on the DVE.
                    ifog = [None, None]
                    c_new = [None, None]
                    for ch in range(CHUNKS):
                        # one fused sigmoid over all 4 gate banks; the cell
                        # gate's weights are pre-scaled x2 so bank 3 yields
                        # g' = sigmoid(2y) with tanh(y) = 2g' - 1
                        ifog[ch] = wpool.tile(
                            [CORE_HID, 4, CB], f16, tag=f"ifog{ch}",
                            name=f"ifog{ch}",
                        )
                        nc.scalar.activation(ifog[ch][:], ps[ch][:], ACT.Sigmoid)
                        i_, f_, o_, g_ = (ifog[ch][:, k] for k in range(4))
                        tg = wpool.tile([CORE_HID, CB], f16, tag=f"tg{ch}",
                                        name=f"tg{ch}")
                        nc.vector.tensor_scalar(
                            tg[:], g_, 2.0, -1.0,
                            mybir.AluOpType.mult, mybir.AluOpType.add,
                        )
                        t1 = wpool.tile([CORE_HID, CB], f16, tag=f"t1{ch}",
                                        name=f"t1{ch}")
                        nc.vector.tensor_mul(t1[:], f_, cs_t[ch][:])  # f*c
                        if ch == 0:
                            t2 = wpool.tile([CORE_HID, CB], f16, tag=f"t2{ch}",
                                            name=f"t2{ch}")
                            nc.vector.tensor_mul(t2[:], i_, tg[:])  # i*tanh(g)
                            c_new[ch] = spool.tile(
                                [CORE_HID, CB], f16, tag=f"c{ch}", name=f"c{ch}"
                            )
                            nc.vector.tensor_add(c_new[ch][:], t1[:], t2[:])
                        else:
                            c_new[ch] = (t1, tg, i_)  # t2+add deferred past h_A
                    for ch in range(CHUNKS):
                        o_ = ifog[ch][:, 2]
                        tc_ = wpool.tile([CORE_HID, CB], f16, tag=f"sc{ch}",
                                         name=f"sc{ch}")
                        nc.scalar.activation(tc_[:], c_new[ch][:], ACT.Tanh)
                        h_new = spool.tile([CORE_HID, CB], f16, tag=f"h{ch}",
                                           name=f"h{ch}")
                        nc.vector.tensor_mul(h_new[:], o_, tc_[:])  # o*tanh(c)
                        hs_t[ch] = h_new
                        if ch + 1 < CHUNKS and isinstance(c_new[ch + 1], tuple):
                            t1, tg, i_b = c_new[ch + 1]
                            t2 = wpool.tile(
                                [CORE_HID, CB], f16, tag=f"t2{ch + 1}",
                                name=f"t2{ch + 1}",
                            )
                            nc.vector.tensor_mul(t2[:], i_b, tg[:])
                            cn = spool.tile(
                                [CORE_HID, CB], f16, tag=f"c{ch + 1}",
                                name=f"c{ch + 1}",
                            )
                            nc.vector.tensor_add(cn[:], t1[:], t2[:])
                            c_new[ch + 1] = cn
                    cs_t = list(c_new)

                # readout partial: oc^T @ h  -> [1, B]
                outsb = wpool.tile([1, B], f32, tag="outsb")
                for ch in range(CHUNKS):
                    pr = ppool.tile([128, 4, CB], f32, tag=f"ps{ch}")
                    nc.tensor.matmul(
                        pr[0:1, 0], oc[:], hs_t[ch][:], start=True, stop=True
                    )
                    nc.vector.tensor_copy(outsb[:, ch * CB : (ch + 1) * CB], pr[0:1, 0])
                nc.sync.dma_start(part_d[:], outsb[:])

    nc.compile()
    return nc


def _pack_inputs(inputs):
    um, vm = _build_masks()
    gates = [
        (inputs["U_i"], inputs["V_i"], inputs["b_i"]),
        (inputs["U_f"], inputs["V_f"], inputs["b_f"]),
        (inputs["U_o"], inputs["V_o"], inputs["b_o"]),
        (inputs["U_c"], inputs["V_c"], inputs["b_c"]),
    ]
    Up = [np.asarray(U, np.float32) * um for U, _, _ in gates]
    Vp = [np.asarray(V, np.float32) * vm for _, V, _ in gates]
    bs = [np.asarray(b, np.float32) for _, _, b in gates]
    x_seq = np.asarray(inputs["x_seq"], np.float32)
    out_coef = np.asarray(inputs["out_coef"], np.float32)

    in_maps = []
    for core in range(N_CORES):
        feats = list(range(4 * core, 4 * core + 4))
        hs = slice(CORE_HID * core, CORE_HID * (core + 1))
        xf = np.ones((5, T * B), np.float32)
        # column index = t*B + b
        xf[0:4] = x_seq[:, :, feats].transpose(2, 1, 0).reshape(4, T * B)
        wu = np.zeros((4, 5, CORE_HID), np.float32)
        wv = np.zeros((4, CORE_HID, CORE_HID), np.float32)
        for g in range(4):
            wu[g, 0:4] = Up[g][feats, hs]
            if core == 0:
                # interaction rows 32,33 multiply x0,x1 -> fold into rows 0,1
                wu[g, 0] += Up[g][32, hs]
                wu[g, 1] += Up[g][33, hs]
            wu[g, 4] = bs[g][hs]
            wv[g] = Vp[g][hs, hs]
        # cell gate (idx 3) pre-scaled x2: tanh(y) = 2*sigmoid(2y) - 1
        wu[3] *= 2.0
        wv[3] *= 2.0
        in_maps.append(
            {
                "xf": xf.astype(np.float16),
                "wu": wu.astype(np.float16),
                "wv": wv.astype(np.float16),
                "oc": np.ascontiguousarray(out_coef[hs]).astype(np.float16),
            }
        )
    return in_maps, Vp, bs, out_coef


def _host_tail(inputs, partials, Vp, bs, out_coef):
    """Bias-only blocks 32,33 (batch-independent scalar) + static MLP +
    final sigmoid. All exact model math, done during unshard."""
    aux = slice(32 * HPF, HID)  # hid 1024:1088
    h = np.zeros(2 * HPF, np.float32)
    cst = np.zeros(2 * HPF, np.float32)
    Va = [V[aux, aux] for V in Vp]
    ba = [b[aux] for b in bs]

    def sig(x):
        return 1.0 / (1.0 + np.exp(-x))

    for _ in range(T):
        i_t = sig(ba[0] + h @ Va[0])
        f_t = sig(ba[1] + h @ Va[1])
        o_t = sig(ba[2] + h @ Va[2])
        g_t = np.tanh(ba[3] + h @ Va[3])
        cst = f_t * cst + i_t * g_t
        h = o_t * np.tanh(cst)
    s_aux = float(h @ out_coef[aux, 0])

    x_stat = np.asarray(inputs["x_stat"], np.float32)
    W1 = np.asarray(inputs["W1"], np.float32)
    b1 = np.asarray(inputs["b1"], np.float32)
    W2 = np.asarray(inputs["W2"], np.float32)
    b2 = np.asarray(inputs["b2"], np.float32)
    hid = np.maximum(x_stat[:, :, None] * W1[None] + b1[None], 0.0)
    mlp = sig(np.einsum("bfk,fk->bf", hid, W2) + b2)
    mlp_part = mlp @ out_coef[HID:, 0]

    z = partials.sum(axis=0) + s_aux + mlp_part + float(np.asarray(inputs["out_bias"])[0])
    return sig(z).astype(np.float32).reshape(B, 1)


def kernel(**inputs):
    from concourse.bass_utils import run_bass_kernel_spmd

    if "nc" not in _CACHE:
        _CACHE["nc"] = _build_program()
    nc = _CACHE["nc"]

    in_maps, Vp, bs, out_coef = _pack_inputs(inputs)
    res = run_bass_kernel_spmd(nc, in_maps, core_ids=list(range(N_CORES)))
    partials = np.stack([res.results[c]["partial"][0] for c in range(N_CORES)])
    return _host_tail(inputs, partials, Vp, bs, out_coef)

